# revision 15
# baseline (speedup 1.0000x reference)
"""GCN(1-layer, normalized adjacency) + dynamic-filter LSTM kernel for TRN2.

Sharding: 8 cores = 4 batches x 2 row-halves (b = core//2, h = core%2).
Per-core j-block order is host-permuted so the core's OWN row-half comes
first (columns 0-15), letting own-half aggregation start before the pair
AllGather delivers the peer half's degree scaling.

Key structure per core:
  - A^T tiles arrive as fp8e4 (0/1 entries are exact): atf8[p, jb, i] =
    A[b, h*2048+i, jb*128+p]; single 8 MB read, SBUF-resident.
  - Degrees via tiny PE matmuls (A^T tile stationary, ones moving),
    accumulated in one PSUM bank; dinv = 1/sqrt(deg+1) is exchanged with
    the pair core by an AllGather (dinv, not deg, so the peer path needs
    no ACT work).
  - aggT[d, i] += Y_jb^T @ atf_jb with Y = dinv_j * X in bf16; +I handled
    by eye-matmul accumulation (after the bank-zeroing first matmul).
  - LSTM (128 steps) runs entirely on PE + ACT: 4 matmuls then 8
    activation ops per step, all [128,1] with scale/bias AP tricks; the
    tanh(c)/h ops of step t-1 are emitted at the head of step t so the
    next matmul waits only 2 ACT ops (software pipelining).
  - Big PE work is emitted after the chain so the Tile list scheduler
    fills chain-idle PE slots with it as DMA chunks arrive.
  - out = sigmoid(dinv_i * aggT^T @ lw), 4 batched output DMAs.
"""
import numpy as np
import ml_dtypes

import concourse.bass as bass
import concourse.bacc as bacc
import concourse.mybir as mybir
import concourse.tile as tile
from concourse import bass_utils

F32 = mybir.dt.float32
BF16 = mybir.dt.bfloat16
FP8 = mybir.dt.float8e4
AF = mybir.ActivationFunctionType
ALU = mybir.AluOpType

B, N, HID = 4, 4096, 128
HALF = N // 2
NB = N // 128
HB = HALF // 128
T = 128

NP_FP8 = mybir.dt.np(FP8)

_CACHED = {}


def build_nc():
    nc = bacc.Bacc("TRN2", target_bir_lowering=False, debug=False, num_devices=8)

    atf8 = nc.dram_tensor("atf8", [128, NB, HALF], FP8, kind="ExternalInput")
    xbf = nc.dram_tensor("xbf", [128, NB, HID], BF16, kind="ExternalInput")
    xdf = nc.dram_tensor("xdf", [128, 128], F32, kind="ExternalInput")
    cwt = nc.dram_tensor("cwt", [3, 128, 128], F32, kind="ExternalInput")
    wihT = nc.dram_tensor("wihT", [128, 512], F32, kind="ExternalInput")
    whhT = nc.dram_tensor("whhT", [128, 512], F32, kind="ExternalInput")
    bS2 = nc.dram_tensor("bS2", [2, 4, 128], F32, kind="ExternalInput")
    onescb = nc.dram_tensor("onescb", [2, 128], F32, kind="ExternalInput")
    eyeb = nc.dram_tensor("eyeb", [128, 128], BF16, kind="ExternalInput")
    eyef = nc.dram_tensor("eyef", [128, 128], F32, kind="ExternalInput")
    onesb = nc.dram_tensor("onesb", [128, 1], BF16, kind="ExternalInput")
    hsel = nc.dram_tensor("hsel", [128, 1], F32, kind="ExternalInput")
    hinv = nc.dram_tensor("hinv", [128, 1], F32, kind="ExternalInput")
    out = nc.dram_tensor("out", [HALF, HID], F32, kind="ExternalOutput")

    with tile.TileContext(nc) as tc:
        with (
            tc.tile_pool(name="const", bufs=1) as cp,
            tc.tile_pool(name="big", bufs=1) as bigp,
            tc.tile_pool(name="step", bufs=4) as stp,
            tc.tile_pool(name="outs", bufs=2) as osp,
            tc.tile_pool(name="psagg", bufs=4, space="PSUM") as psagg,
            tc.tile_pool(name="pslstm", bufs=1, space="PSUM") as pslstm,
            tc.tile_pool(name="psmisc", bufs=2, space="PSUM") as psmisc,
            tc.tile_pool(name="psdeg", bufs=1, space="PSUM") as psdeg,
            tc.tile_pool(name="dram", bufs=1, space="DRAM") as dram,
        ):
            # ---------- small loads (LSTM-critical first) ----------
            dfpad = cp.tile([128, 130], F32, tag="dfpad")
            cwt_sb = cp.tile([128, 384], F32, tag="cwt")
            wihT_sb = cp.tile([128, 512], F32, tag="wihT")
            whhT_sb = cp.tile([128, 512], F32, tag="whhT")
            bS2_sb = cp.tile([2, 4, 128], F32, tag="bS2")
            ocb_sb = cp.tile([2, 128], F32, tag="ocb")
            eyeb_sb = cp.tile([128, 128], BF16, tag="eyeb")
            eyef_sb = cp.tile([128, 128], F32, tag="eyef")
            ones_sb = cp.tile([128, 1], BF16, tag="ones")
            hsel_sb = cp.tile([128, 1], F32, tag="hsel")
            hinv_sb = cp.tile([128, 1], F32, tag="hinv")
            nc.vector.memset(dfpad[:], 0.0)
            nc.sync.dma_start(dfpad[:, 1:129], xdf[:])
            for k in range(3):
                nc.sync.dma_start(cwt_sb[:, k * 128:(k + 1) * 128], cwt[k])
            nc.sync.dma_start(wihT_sb[:], wihT[:])
            nc.sync.dma_start(bS2_sb[:], bS2[:])
            nc.sync.dma_start(ocb_sb[:], onescb[:])
            nc.sync.dma_start(whhT_sb[:], whhT[:])
            nc.sync.dma_start(eyeb_sb[:], eyeb[:])
            nc.sync.dma_start(eyef_sb[:], eyef[:])
            nc.sync.dma_start(ones_sb[:], onesb[:])
            nc.sync.dma_start(hsel_sb[:], hsel[:])
            nc.sync.dma_start(hinv_sb[:], hinv[:])

            # big A^T load: 16 chunks of 2 j-blocks (own half = chunks 0-7)
            atf_sb = bigp.tile([128, NB, HALF], FP8, tag="atf")
            for c in range(16):
                nc.sync.dma_start(
                    atf_sb[:, c * 2:(c + 1) * 2, :], atf8[:, c * 2:(c + 1) * 2, :]
                )
            xbf_sb = bigp.tile([128, NB, HID], BF16, tag="xbf")
            nc.sync.dma_start(xbf_sb[:], xbf[:])

            # ---------- conv -> dynT ----------
            dyn_ps = psmisc.tile([128, 128], F32, tag="mm128")
            for k in range(3):
                nc.tensor.matmul(
                    dyn_ps[:], dfpad[:, k:k + 128], cwt_sb(k * 128, (k + 1) * 128),
                    start=(k == 0), stop=(k == 2),
                )
            dynT_sb = cp.tile([128, 128], BF16, tag="dynT")
            nc.vector.tensor_copy(dynT_sb[:], dyn_ps[:])

            # ---------- Zx[u, t*4+g]: matmul + rank-2 bias fold ----------
            # split: t<32 computed up front (fast LSTM start), t>=32 interleaved
            ZxA_sb = bigp.tile([128, 32 * 4], F32, tag="ZxA")
            ZxB_sb = bigp.tile([128, 96 * 4], F32, tag="ZxB")
            ZxA_g = ZxA_sb[:].rearrange("p (t g) -> p g t", g=4)
            ZxB_g = ZxB_sb[:].rearrange("p (t g) -> p g t", g=4)
            for g4 in range(4):
                zx_t = psmisc.tile([128, 128], F32, tag="mm128")
                zx_ps = zx_t[:, 0:32]
                nc.tensor.matmul(
                    zx_ps[:], wihT_sb(g4 * 128, (g4 + 1) * 128), dynT_sb[:, 0:32],
                    start=True, stop=False,
                )
                nc.tensor.matmul(
                    zx_ps[:], bS2_sb(g4), ocb_sb(0, 32),
                    start=False, stop=True,
                )
                nc.vector.tensor_copy(ZxA_g[:, g4], zx_ps[:])

            def zx_rest_ops():
                for g4 in range(4):
                    ps_t = psmisc.tile([128, 128], F32, tag="mm128")
                    ps = ps_t[:, 0:96]
                    yield lambda g4=g4, ps=ps: nc.tensor.matmul(
                        ps[:], wihT_sb(g4 * 128, (g4 + 1) * 128), dynT_sb[:, 32:128],
                        start=True, stop=False,
                    )
                    yield lambda g4=g4, ps=ps: nc.tensor.matmul(
                        ps[:], bS2_sb(g4), ocb_sb(32, 128),
                        start=False, stop=True,
                    )
                    yield lambda g4=g4, ps=ps: nc.vector.tensor_copy(ZxB_g[:, g4], ps[:])

            def zx_bias(t, g):
                if t < 32:
                    return ZxA_sb[:, t * 4 + g:t * 4 + g + 1]
                return ZxB_sb[:, (t - 32) * 4 + g:(t - 32) * 4 + g + 1]

            # ---------- persistent state ----------
            H_all = bigp.tile([128, T + 1], F32, tag="H")
            c_pp = cp.tile([128, 2], F32, tag="cpp")
            v_sb = cp.tile([128, 1], F32, tag="v")
            tc_sb = cp.tile([128, 1], F32, tag="tc")
            nc.vector.memset(H_all[:, 0:1], 0.0)
            nc.vector.memset(c_pp[:], 0.0)

            deg_psT = psdeg.tile([128, 128], F32, tag="degps")
            Y_sb = bigp.tile([128, NB, HID], BF16, tag="Y")
            aggT_sb = bigp.tile([128, 4, 512], F32, tag="aggT")
            dinv_own = cp.tile([128, HB], F32, tag="dinvo")
            dinv_peer = cp.tile([128, HB], F32, tag="dinvp")
            deg_own = cp.tile([128, HB], F32, tag="dego")
            lw_sb = cp.tile([128, 128], F32, tag="lw")
            g0_sb = cp.tile([128, HB], F32, tag="g0")
            g1_sb = cp.tile([128, HB], F32, tag="g1")

            cc_in = dram.tile([128, HB], F32)
            cc_out = dram.tile([2, 128, HB], F32)

            agg_ps = [None] * 4

            # ---------- big-PE generators ----------
            def deg_chunk_ops(c):
                # PSUM zero-region is the whole 2KB bank: exactly ONE start=True
                # (first op) zeroes the bank; every later op accumulates.
                for jb in range(c * 2, (c + 1) * 2):
                    for ib in range(HB):
                        yield lambda jb=jb, ib=ib: nc.tensor.matmul(
                            deg_psT[:, ib:ib + 1],
                            atf_sb[:, jb, ib * 128:(ib + 1) * 128],
                            ones_sb(),
                            start=(jb == 0 and ib == 0),
                            stop=(jb == NB - 1 and ib == HB - 1),
                            skip_group_check=True,
                        )

            def own_agg_ops():
                for ic in range(4):
                    ps = psagg.tile([128, 512], F32, tag="agg")
                    agg_ps[ic] = ps
                    # first matmul zeroes the whole bank (start=True); later ops
                    # (incl. +I eye matmuls) accumulate. N=256 halves keep PE ops
                    # short so chain matmuls are not blocked behind long ops.
                    for hf in range(4):
                        yield lambda ic=ic, hf=hf, ps=ps: nc.tensor.matmul(
                            ps[:, hf * 128:(hf + 1) * 128], Y_sb[:, 0, :],
                            atf_sb[:, 0, ic * 512 + hf * 128:ic * 512 + (hf + 1) * 128],
                            start=(hf == 0), stop=False, skip_group_check=True,
                        )
                    for s in range(4):
                        yield lambda ic=ic, s=s, ps=ps: nc.tensor.matmul(
                            ps[:, s * 128:(s + 1) * 128],
                            Y_sb[:, ic * 4 + s, :], eyeb_sb(),
                            start=False, stop=False, skip_group_check=True,
                        )
                    for jb in range(1, HB):
                        for hf in range(4):
                            yield lambda jb=jb, ic=ic, hf=hf, ps=ps: nc.tensor.matmul(
                                ps[:, hf * 128:(hf + 1) * 128], Y_sb[:, jb, :],
                                atf_sb[:, jb, ic * 512 + hf * 128:ic * 512 + (hf + 1) * 128],
                                start=False, stop=False, skip_group_check=True,
                            )

            def peer_agg_ops():
                for ic in range(4):
                    ps = agg_ps[ic]
                    for jb in range(HB, NB):
                        for hf in range(4):
                            yield lambda jb=jb, ic=ic, hf=hf, ps=ps: nc.tensor.matmul(
                                ps[:, hf * 128:(hf + 1) * 128], Y_sb[:, jb, :],
                                atf_sb[:, jb, ic * 512 + hf * 128:ic * 512 + (hf + 1) * 128],
                                start=False, stop=(jb == NB - 1 and hf == 3),
                                skip_group_check=True,
                            )
                    yield lambda ic=ic, ps=ps: nc.vector.tensor_copy(
                        aggT_sb[:, ic, :], ps[:]
                    )

            def emit_deg_collect():
                nc.vector.tensor_copy(deg_own[:], deg_psT[:, 0:HB])
                nc.vector.tensor_scalar_add(deg_own[:], deg_own[:], 1.0)

            def emit_own_dinv():
                # dinv_own = 1/sqrt(deg+1); collective gathers dinv directly
                sq = stp.tile([128, HB], F32, tag="sq")
                nc.scalar.activation(sq[:], deg_own[:], AF.Sqrt)
                nc.vector.reciprocal(dinv_own[:], sq[:])
                nc.sync.dma_start(cc_in[:], dinv_own[:])
                nc.gpsimd.collective_compute(
                    "AllGather", ALU.bypass,
                    replica_groups=[[0, 1], [2, 3], [4, 5], [6, 7]],
                    ins=[cc_in.opt()], outs=[cc_out.opt()],
                )
                for k in range(HB):
                    nc.vector.tensor_scalar_mul(
                        Y_sb[:, k, :], xbf_sb[:, k, :], dinv_own[:, k:k + 1]
                    )

            def emit_peer_dinv():
                nc.sync.dma_start(g0_sb[:], cc_out[0])
                nc.sync.dma_start(g1_sb[:], cc_out[1])
                # peer = hinv*g1 + hsel*g0   (h=0 -> peer is member1)
                t1 = stp.tile([128, HB], F32, tag="t1")
                nc.vector.tensor_scalar_mul(t1[:], g1_sb[:], hinv_sb())
                nc.vector.tensor_scalar_mul(dinv_peer[:], g0_sb[:], hsel_sb())
                nc.vector.tensor_tensor(dinv_peer[:], dinv_peer[:], t1[:], op=ALU.add)
                for k in range(HB):
                    nc.vector.tensor_scalar_mul(
                        Y_sb[:, HB + k, :], xbf_sb[:, HB + k, :], dinv_peer[:, k:k + 1]
                    )

            # Zx for steps 32..127: must be EMITTED before the chain reads it
            # (Tile dependencies follow emission order), but with a priority
            # bump so engines only run it in chain-idle slots.
            _sv = tc.cur_priority
            tc.cur_priority = _sv + 1_000_000
            for op in zx_rest_ops():
                op()
            tc.cur_priority = _sv

            # ---------- main loop: pure chain (lowest priorities) ----------
            ga_prev = None
            for t in range(T):
                # head: finish step t-1 (tanh(c), h) so zp_t waits only 2 ACT ops
                if t > 0:
                    cprev = c_pp[:, t % 2:t % 2 + 1]
                    nc.scalar.activation(tc_sb[:], cprev, AF.Tanh)
                    nc.scalar.activation(H_all[:, t:t + 1], tc_sb[:], AF.Copy, scale=ga_prev[:, 2:3])
                zp = pslstm.tile([128, 4], F32, tag="zp")
                for g in range(4):
                    nc.tensor.matmul(
                        zp[:, g:g + 1], whhT_sb(g * 128, (g + 1) * 128),
                        H_all[:, t:t + 1], start=True, stop=True,
                    )
                ga = stp.tile([128, 4], F32, tag="ga")
                nc.scalar.activation(ga[:, 0:1], zp[:, 0:1], AF.Sigmoid, bias=zx_bias(t, 0))
                nc.scalar.activation(ga[:, 3:4], zp[:, 3:4], AF.Tanh, bias=zx_bias(t, 3))
                nc.scalar.activation(v_sb[:], ga[:, 3:4], AF.Copy, scale=ga[:, 0:1])
                nc.scalar.activation(ga[:, 1:2], zp[:, 1:2], AF.Sigmoid, bias=zx_bias(t, 1))
                cr = c_pp[:, t % 2:t % 2 + 1]
                cw = c_pp[:, (t + 1) % 2:(t + 1) % 2 + 1]
                nc.scalar.activation(cw, cr, AF.Identity, scale=ga[:, 1:2], bias=v_sb[:])
                nc.scalar.activation(ga[:, 2:3], zp[:, 2:3], AF.Sigmoid, bias=zx_bias(t, 2))
                ga_prev = ga

            nc.scalar.activation(tc_sb[:], c_pp[:, T % 2:T % 2 + 1], AF.Tanh)
            nc.scalar.activation(H_all[:, T:T + 1], tc_sb[:], AF.Copy, scale=ga_prev[:, 2:3])

            # big-PE work after the chain: higher priority numbers, so the
            # list scheduler runs it only in chain-idle slots, as data arrives.
            for c in range(16):
                for op in deg_chunk_ops(c):
                    op()
            emit_deg_collect()
            emit_own_dinv()
            for op in own_agg_ops():
                op()
            emit_peer_dinv()
            for op in peer_agg_ops():
                op()

            # ---------- lw = H[:, 1:]^T ----------
            lw_ps = psmisc.tile([128, 128], F32, tag="mm128")
            nc.tensor.transpose(lw_ps[:], H_all[:, 1:T + 1], eyef_sb())
            nc.vector.tensor_copy(lw_sb[:], lw_ps[:])

            # ---------- final: 3-way psum rotation (psmisc x2 + freed psdeg) ----------
            for ic in range(4):
                o_sb = osp.tile([128, 4, 128], F32, tag="osb")
                for s in range(4):
                    ib = ic * 4 + s
                    if (ic * 4 + s) % 3 == 2:
                        fin_t = deg_psT
                    else:
                        fin_t = psmisc.tile([128, 128], F32, tag="mm128")
                    out_ap = fin_t[:]
                    nc.tensor.matmul(
                        out_ap, aggT_sb[:, ic, s * 128:(s + 1) * 128], lw_sb[:],
                        start=True, stop=True,
                    )
                    nc.scalar.activation(
                        o_sb[:, s, :], out_ap, AF.Sigmoid,
                        scale=dinv_own[:, ib:ib + 1],
                    )
                nc.sync.dma_start(
                    out[ic * 512:(ic + 1) * 512, :].rearrange("(s p) d -> p s d", p=128),
                    o_sb[:],
                )
    nc.compile()
    return nc


PERM = np.concatenate([np.arange(0, 128), np.arange(128, 256),
                       np.arange(384, 512), np.arange(256, 384)])


def make_in_maps(node_embedding, adjacency_matrix, conv_w, conv_b, w_ih, w_hh, b_ih, b_hh):
    X = np.asarray(node_embedding, dtype=np.float32)
    A = np.asarray(adjacency_matrix, dtype=np.float32)
    wih_p = np.asarray(w_ih, dtype=np.float32)[PERM]
    whh_p = np.asarray(w_hh, dtype=np.float32)[PERM]
    bias_p = (np.asarray(b_ih, dtype=np.float32) + np.asarray(b_hh, dtype=np.float32))[PERM]
    S = wih_p.sum(axis=1)

    cwt = np.asarray(conv_w, dtype=np.float32).transpose(2, 1, 0)  # [3,128,128]
    packg = np.zeros((2, 640), np.float32)
    packg[:, 0:512] = np.stack([bias_p, S]).reshape(2, 4, 128).reshape(2, 512)
    packg[0, 512:640] = 1.0
    packg[1, 512:640] = np.asarray(conv_b, np.float32)

    packb = np.zeros((128, 641), ml_dtypes.bfloat16)
    packb[:, 0:128] = np.eye(128, dtype=ml_dtypes.bfloat16)
    packb[:, 128] = 1.0
    packb[:, 129:641] = wih_p.T.astype(ml_dtypes.bfloat16)

    zeros1 = np.zeros((128,), np.float32)
    ones1 = np.ones((128,), np.float32)

    in_maps = []
    for c in range(8):
        b, h = c // 2, c % 2
        packc = np.zeros((128, 512), np.float32)
        packc[:, 0:128] = X[b, N - HID:, :]
        packc[:, 128:512] = np.concatenate([cwt[0], cwt[1], cwt[2]], axis=1)
        packc = packc.astype(ml_dtypes.bfloat16)
        packf = np.zeros((128, 642), np.float32)
        packf[:, 0:512] = whh_p.T
        packf[:, 512:640] = np.eye(128, dtype=np.float32)
        packf[:, 640] = ones1 if h == 1 else zeros1
        packf[:, 641] = zeros1 if h == 1 else ones1

        jorder = np.concatenate([np.arange(h * HB, (h + 1) * HB),
                                 np.arange((1 - h) * HB, (2 - h) * HB)])
        Ah = A[b, h * HALF:(h + 1) * HALF, :]
        AT = np.ascontiguousarray(Ah.T)
        atf = AT.reshape(NB, 128, HALF)[jorder].transpose(1, 0, 2)
        xb = X[b].reshape(NB, 128, HID)[jorder].transpose(1, 0, 2)
        m = {
            "packc": packc,
            "packf": packf,
            "packb": packb,
            "packg": packg,
            "atf8": np.ascontiguousarray(atf).astype(NP_FP8),
            "xbf": np.ascontiguousarray(xb).astype(ml_dtypes.bfloat16),
        }
        in_maps.append(m)
    return in_maps


def kernel(node_embedding, adjacency_matrix, conv_w, conv_b, w_ih, w_hh, b_ih, b_hh):
    if "nc" not in _CACHED:
        _CACHED["nc"] = build_nc()
    nc = _CACHED["nc"]
    in_maps = make_in_maps(node_embedding, adjacency_matrix, conv_w, conv_b,
                           w_ih, w_hh, b_ih, b_hh)
    _CACHED["in_maps"] = in_maps
    res = bass_utils.run_bass_kernel_spmd(nc, in_maps, core_ids=list(range(8)))
    out = np.empty((B, N, HID), np.float32)
    for c in range(8):
        b, h = c // 2, c % 2
        out[b, h * HALF:(h + 1) * HALF, :] = res.results[c]["out"]
    return out


# revision 16
# speedup vs baseline: 1.0005x; 1.0005x over previous
"""GCN(1-layer, normalized adjacency) + dynamic-filter LSTM kernel for TRN2.

Sharding: 8 cores = 4 batches x 2 row-halves (b = core//2, h = core%2).
Per-core j-block order is host-permuted so the core's OWN row-half comes
first (columns 0-15), letting own-half aggregation start before the pair
AllGather delivers the peer half's degree scaling.

Key structure per core:
  - A^T tiles arrive as fp8e4 (0/1 entries are exact): atf8[p, jb, i] =
    A[b, h*2048+i, jb*128+p]; single 8 MB read, SBUF-resident.
  - Degrees via tiny PE matmuls (A^T tile stationary, ones moving),
    accumulated in one PSUM bank; dinv = 1/sqrt(deg+1) is exchanged with
    the pair core by an AllGather (dinv, not deg, so the peer path needs
    no ACT work).
  - aggT[d, i] += Y_jb^T @ atf_jb with Y = dinv_j * X in bf16; +I handled
    by eye-matmul accumulation (after the bank-zeroing first matmul).
  - LSTM (128 steps) runs entirely on PE + ACT: 4 matmuls then 8
    activation ops per step, all [128,1] with scale/bias AP tricks; the
    tanh(c)/h ops of step t-1 are emitted at the head of step t so the
    next matmul waits only 2 ACT ops (software pipelining).
  - Big PE work is emitted after the chain so the Tile list scheduler
    fills chain-idle PE slots with it as DMA chunks arrive.
  - out = sigmoid(dinv_i * aggT^T @ lw), 4 batched output DMAs.
"""
import numpy as np
import ml_dtypes

import concourse.bass as bass
import concourse.bacc as bacc
import concourse.mybir as mybir
import concourse.tile as tile
from concourse import bass_utils

F32 = mybir.dt.float32
BF16 = mybir.dt.bfloat16
FP8 = mybir.dt.float8e4
AF = mybir.ActivationFunctionType
ALU = mybir.AluOpType

B, N, HID = 4, 4096, 128
HALF = N // 2
NB = N // 128
HB = HALF // 128
T = 128

NP_FP8 = mybir.dt.np(FP8)

_CACHED = {}


def build_nc():
    nc = bacc.Bacc("TRN2", target_bir_lowering=False, debug=False, num_devices=8)

    atf8 = nc.dram_tensor("atf8", [128, NB, HALF], FP8, kind="ExternalInput")
    xbf = nc.dram_tensor("xbf", [128, NB, HID], BF16, kind="ExternalInput")
    xdf = nc.dram_tensor("xdf", [128, 128], F32, kind="ExternalInput")
    cwt = nc.dram_tensor("cwt", [3, 128, 128], F32, kind="ExternalInput")
    wihT = nc.dram_tensor("wihT", [128, 512], F32, kind="ExternalInput")
    whhT = nc.dram_tensor("whhT", [128, 512], F32, kind="ExternalInput")
    bS2 = nc.dram_tensor("bS2", [2, 4, 128], F32, kind="ExternalInput")
    onescb = nc.dram_tensor("onescb", [2, 128], F32, kind="ExternalInput")
    eyeb = nc.dram_tensor("eyeb", [128, 128], BF16, kind="ExternalInput")
    eyef = nc.dram_tensor("eyef", [128, 128], F32, kind="ExternalInput")
    onesb = nc.dram_tensor("onesb", [128, 1], BF16, kind="ExternalInput")
    hsel = nc.dram_tensor("hsel", [128, 1], F32, kind="ExternalInput")
    hinv = nc.dram_tensor("hinv", [128, 1], F32, kind="ExternalInput")
    out = nc.dram_tensor("out", [HALF, HID], F32, kind="ExternalOutput")

    with tile.TileContext(nc) as tc:
        with (
            tc.tile_pool(name="const", bufs=1) as cp,
            tc.tile_pool(name="big", bufs=1) as bigp,
            tc.tile_pool(name="step", bufs=4) as stp,
            tc.tile_pool(name="outs", bufs=2) as osp,
            tc.tile_pool(name="psagg", bufs=4, space="PSUM") as psagg,
            tc.tile_pool(name="pslstm", bufs=1, space="PSUM") as pslstm,
            tc.tile_pool(name="psmisc", bufs=2, space="PSUM") as psmisc,
            tc.tile_pool(name="psdeg", bufs=1, space="PSUM") as psdeg,
            tc.tile_pool(name="dram", bufs=1, space="DRAM") as dram,
        ):
            # ---------- small loads (LSTM-critical first) ----------
            dfpad = cp.tile([128, 130], F32, tag="dfpad")
            cwt_sb = cp.tile([128, 384], F32, tag="cwt")
            wihT_sb = cp.tile([128, 512], F32, tag="wihT")
            whhT_sb = cp.tile([128, 512], F32, tag="whhT")
            bS2_sb = cp.tile([2, 4, 128], F32, tag="bS2")
            ocb_sb = cp.tile([2, 128], F32, tag="ocb")
            eyeb_sb = cp.tile([128, 128], BF16, tag="eyeb")
            eyef_sb = cp.tile([128, 128], F32, tag="eyef")
            ones_sb = cp.tile([128, 1], BF16, tag="ones")
            hsel_sb = cp.tile([128, 1], F32, tag="hsel")
            hinv_sb = cp.tile([128, 1], F32, tag="hinv")
            nc.vector.memset(dfpad[:], 0.0)
            nc.sync.dma_start(dfpad[:, 1:129], xdf[:])
            for k in range(3):
                nc.sync.dma_start(cwt_sb[:, k * 128:(k + 1) * 128], cwt[k])
            nc.sync.dma_start(wihT_sb[:], wihT[:])
            nc.sync.dma_start(bS2_sb[:], bS2[:])
            nc.sync.dma_start(ocb_sb[:], onescb[:])
            nc.sync.dma_start(whhT_sb[:], whhT[:])
            nc.sync.dma_start(eyeb_sb[:], eyeb[:])
            nc.sync.dma_start(eyef_sb[:], eyef[:])
            nc.sync.dma_start(ones_sb[:], onesb[:])
            nc.sync.dma_start(hsel_sb[:], hsel[:])
            nc.sync.dma_start(hinv_sb[:], hinv[:])

            # big A^T load: 16 chunks of 2 j-blocks (own half = chunks 0-7)
            atf_sb = bigp.tile([128, NB, HALF], FP8, tag="atf")
            for c in range(16):
                nc.sync.dma_start(
                    atf_sb[:, c * 2:(c + 1) * 2, :], atf8[:, c * 2:(c + 1) * 2, :]
                )
            xbf_sb = bigp.tile([128, NB, HID], BF16, tag="xbf")
            nc.sync.dma_start(xbf_sb[:], xbf[:])

            # ---------- conv -> dynT ----------
            dyn_ps = psmisc.tile([128, 128], F32, tag="mm128")
            for k in range(3):
                nc.tensor.matmul(
                    dyn_ps[:], dfpad[:, k:k + 128], cwt_sb(k * 128, (k + 1) * 128),
                    start=(k == 0), stop=(k == 2),
                )
            dynT_sb = cp.tile([128, 128], BF16, tag="dynT")
            nc.vector.tensor_copy(dynT_sb[:], dyn_ps[:])

            # ---------- Zx[u, t*4+g]: matmul + rank-2 bias fold ----------
            # split: t<32 computed up front (fast LSTM start), t>=32 interleaved
            ZxA_sb = bigp.tile([128, 32 * 4], F32, tag="ZxA")
            ZxB_sb = bigp.tile([128, 96 * 4], F32, tag="ZxB")
            ZxA_g = ZxA_sb[:].rearrange("p (t g) -> p g t", g=4)
            ZxB_g = ZxB_sb[:].rearrange("p (t g) -> p g t", g=4)
            for g4 in range(4):
                zx_t = psmisc.tile([128, 128], F32, tag="mm128")
                zx_ps = zx_t[:, 0:32]
                nc.tensor.matmul(
                    zx_ps[:], wihT_sb(g4 * 128, (g4 + 1) * 128), dynT_sb[:, 0:32],
                    start=True, stop=False,
                )
                nc.tensor.matmul(
                    zx_ps[:], bS2_sb(g4), ocb_sb(0, 32),
                    start=False, stop=True,
                )
                nc.vector.tensor_copy(ZxA_g[:, g4], zx_ps[:])

            def zx_rest_ops():
                for g4 in range(4):
                    ps_t = psmisc.tile([128, 128], F32, tag="mm128")
                    ps = ps_t[:, 0:96]
                    yield lambda g4=g4, ps=ps: nc.tensor.matmul(
                        ps[:], wihT_sb(g4 * 128, (g4 + 1) * 128), dynT_sb[:, 32:128],
                        start=True, stop=False,
                    )
                    yield lambda g4=g4, ps=ps: nc.tensor.matmul(
                        ps[:], bS2_sb(g4), ocb_sb(32, 128),
                        start=False, stop=True,
                    )
                    yield lambda g4=g4, ps=ps: nc.vector.tensor_copy(ZxB_g[:, g4], ps[:])

            def zx_bias(t, g):
                if t < 32:
                    return ZxA_sb[:, t * 4 + g:t * 4 + g + 1]
                return ZxB_sb[:, (t - 32) * 4 + g:(t - 32) * 4 + g + 1]

            # ---------- persistent state ----------
            H_all = bigp.tile([128, T + 1], F32, tag="H")
            c_pp = cp.tile([128, 2], F32, tag="cpp")
            v_sb = cp.tile([128, 1], F32, tag="v")
            tc_sb = cp.tile([128, 1], F32, tag="tc")
            nc.vector.memset(H_all[:, 0:1], 0.0)
            nc.vector.memset(c_pp[:], 0.0)

            deg_psT = psdeg.tile([128, 128], F32, tag="degps")
            Y_sb = bigp.tile([128, NB, HID], BF16, tag="Y")
            aggT_sb = bigp.tile([128, 4, 512], F32, tag="aggT")
            dinv_own = cp.tile([128, HB], F32, tag="dinvo")
            dinv_peer = cp.tile([128, HB], F32, tag="dinvp")
            deg_own = cp.tile([128, HB], F32, tag="dego")
            lw_sb = cp.tile([128, 128], F32, tag="lw")
            g01_sb = cp.tile([128, 2, HB], F32, tag="g01")

            cc_in = dram.tile([128, HB], F32)
            cc_out = dram.tile([2, 128, HB], F32)

            agg_ps = [None] * 4

            # ---------- big-PE generators ----------
            def deg_chunk_ops(c):
                # PSUM zero-region is the whole 2KB bank: exactly ONE start=True
                # (first op) zeroes the bank; every later op accumulates.
                for jb in range(c * 2, (c + 1) * 2):
                    for ib in range(HB):
                        yield lambda jb=jb, ib=ib: nc.tensor.matmul(
                            deg_psT[:, ib:ib + 1],
                            atf_sb[:, jb, ib * 128:(ib + 1) * 128],
                            ones_sb(),
                            start=(jb == 0 and ib == 0),
                            stop=(jb == NB - 1 and ib == HB - 1),
                            skip_group_check=True,
                        )

            def own_agg_ops():
                for ic in range(4):
                    ps = psagg.tile([128, 512], F32, tag="agg")
                    agg_ps[ic] = ps
                    # first matmul zeroes the whole bank (start=True); later ops
                    # (incl. +I eye matmuls) accumulate. N=256 halves keep PE ops
                    # short so chain matmuls are not blocked behind long ops.
                    for hf in range(4):
                        yield lambda ic=ic, hf=hf, ps=ps: nc.tensor.matmul(
                            ps[:, hf * 128:(hf + 1) * 128], Y_sb[:, 0, :],
                            atf_sb[:, 0, ic * 512 + hf * 128:ic * 512 + (hf + 1) * 128],
                            start=(hf == 0), stop=False, skip_group_check=True,
                        )
                    for s in range(4):
                        yield lambda ic=ic, s=s, ps=ps: nc.tensor.matmul(
                            ps[:, s * 128:(s + 1) * 128],
                            Y_sb[:, ic * 4 + s, :], eyeb_sb(),
                            start=False, stop=False, skip_group_check=True,
                        )
                    for jb in range(1, HB):
                        for hf in range(4):
                            yield lambda jb=jb, ic=ic, hf=hf, ps=ps: nc.tensor.matmul(
                                ps[:, hf * 128:(hf + 1) * 128], Y_sb[:, jb, :],
                                atf_sb[:, jb, ic * 512 + hf * 128:ic * 512 + (hf + 1) * 128],
                                start=False, stop=False, skip_group_check=True,
                            )

            def peer_agg_ops():
                for ic in range(4):
                    ps = agg_ps[ic]
                    for jb in range(HB, NB):
                        for hf in range(4):
                            yield lambda jb=jb, ic=ic, hf=hf, ps=ps: nc.tensor.matmul(
                                ps[:, hf * 128:(hf + 1) * 128], Y_sb[:, jb, :],
                                atf_sb[:, jb, ic * 512 + hf * 128:ic * 512 + (hf + 1) * 128],
                                start=False, stop=(jb == NB - 1 and hf == 3),
                                skip_group_check=True,
                            )
                    yield lambda ic=ic, ps=ps: nc.vector.tensor_copy(
                        aggT_sb[:, ic, :], ps[:]
                    )

            def emit_deg_collect():
                nc.vector.tensor_copy(deg_own[:], deg_psT[:, 0:HB])
                nc.vector.tensor_scalar_add(deg_own[:], deg_own[:], 1.0)

            def emit_own_dinv():
                # dinv_own = 1/sqrt(deg+1); collective gathers dinv directly
                sq = stp.tile([128, HB], F32, tag="sq")
                nc.scalar.activation(sq[:], deg_own[:], AF.Sqrt)
                nc.vector.reciprocal(dinv_own[:], sq[:])
                nc.sync.dma_start(cc_in[:], dinv_own[:])
                nc.gpsimd.collective_compute(
                    "AllGather", ALU.bypass,
                    replica_groups=[[0, 1], [2, 3], [4, 5], [6, 7]],
                    ins=[cc_in.opt()], outs=[cc_out.opt()],
                )
                for k in range(HB):
                    nc.vector.tensor_scalar_mul(
                        Y_sb[:, k, :], xbf_sb[:, k, :], dinv_own[:, k:k + 1]
                    )

            def emit_peer_dinv():
                # single DMA for both gathered members (saves one fixed DMA cost)
                nc.sync.dma_start(g01_sb[:], cc_out[:].rearrange("m p c -> p m c"))
                # peer = hinv*member1 + hsel*member0   (h=0 -> peer is member1)
                t1 = stp.tile([128, HB], F32, tag="t1")
                nc.vector.tensor_scalar_mul(t1[:], g01_sb[:, 1, :], hinv_sb())
                nc.vector.tensor_scalar_mul(dinv_peer[:], g01_sb[:, 0, :], hsel_sb())
                nc.vector.tensor_tensor(dinv_peer[:], dinv_peer[:], t1[:], op=ALU.add)
                for k in range(HB):
                    nc.vector.tensor_scalar_mul(
                        Y_sb[:, HB + k, :], xbf_sb[:, HB + k, :], dinv_peer[:, k:k + 1]
                    )

            # Zx for steps 32..127: must be EMITTED before the chain reads it
            # (Tile dependencies follow emission order), but with a priority
            # bump so engines only run it in chain-idle slots.
            _sv = tc.cur_priority
            tc.cur_priority = _sv + 1_000_000
            for op in zx_rest_ops():
                op()
            tc.cur_priority = _sv

            # ---------- main loop: pure chain (lowest priorities) ----------
            ga_prev = None
            for t in range(T):
                # head: finish step t-1 (tanh(c), h) so zp_t waits only 2 ACT ops
                if t > 0:
                    cprev = c_pp[:, t % 2:t % 2 + 1]
                    nc.scalar.activation(tc_sb[:], cprev, AF.Tanh)
                    nc.scalar.activation(H_all[:, t:t + 1], tc_sb[:], AF.Copy, scale=ga_prev[:, 2:3])
                zp = pslstm.tile([128, 4], F32, tag="zp")
                for g in range(4):
                    nc.tensor.matmul(
                        zp[:, g:g + 1], whhT_sb(g * 128, (g + 1) * 128),
                        H_all[:, t:t + 1], start=True, stop=True,
                    )
                ga = stp.tile([128, 4], F32, tag="ga")
                nc.scalar.activation(ga[:, 0:1], zp[:, 0:1], AF.Sigmoid, bias=zx_bias(t, 0))
                nc.scalar.activation(ga[:, 3:4], zp[:, 3:4], AF.Tanh, bias=zx_bias(t, 3))
                nc.scalar.activation(v_sb[:], ga[:, 3:4], AF.Copy, scale=ga[:, 0:1])
                nc.scalar.activation(ga[:, 1:2], zp[:, 1:2], AF.Sigmoid, bias=zx_bias(t, 1))
                cr = c_pp[:, t % 2:t % 2 + 1]
                cw = c_pp[:, (t + 1) % 2:(t + 1) % 2 + 1]
                nc.scalar.activation(cw, cr, AF.Identity, scale=ga[:, 1:2], bias=v_sb[:])
                nc.scalar.activation(ga[:, 2:3], zp[:, 2:3], AF.Sigmoid, bias=zx_bias(t, 2))
                ga_prev = ga

            nc.scalar.activation(tc_sb[:], c_pp[:, T % 2:T % 2 + 1], AF.Tanh)
            nc.scalar.activation(H_all[:, T:T + 1], tc_sb[:], AF.Copy, scale=ga_prev[:, 2:3])

            # big-PE work after the chain: higher priority numbers, so the
            # list scheduler runs it only in chain-idle slots, as data arrives.
            for c in range(16):
                for op in deg_chunk_ops(c):
                    op()
            emit_deg_collect()
            emit_own_dinv()
            for op in own_agg_ops():
                op()
            emit_peer_dinv()
            for op in peer_agg_ops():
                op()

            # ---------- lw = H[:, 1:]^T ----------
            lw_ps = psmisc.tile([128, 128], F32, tag="mm128")
            nc.tensor.transpose(lw_ps[:], H_all[:, 1:T + 1], eyef_sb())
            nc.vector.tensor_copy(lw_sb[:], lw_ps[:])

            # ---------- final: 3-way psum rotation (psmisc x2 + freed psdeg) ----------
            for ic in range(4):
                o_sb = osp.tile([128, 4, 128], F32, tag="osb")
                for s in range(4):
                    ib = ic * 4 + s
                    if (ic * 4 + s) % 3 == 2:
                        fin_t = deg_psT
                    else:
                        fin_t = psmisc.tile([128, 128], F32, tag="mm128")
                    out_ap = fin_t[:]
                    nc.tensor.matmul(
                        out_ap, aggT_sb[:, ic, s * 128:(s + 1) * 128], lw_sb[:],
                        start=True, stop=True,
                    )
                    nc.scalar.activation(
                        o_sb[:, s, :], out_ap, AF.Sigmoid,
                        scale=dinv_own[:, ib:ib + 1],
                    )
                nc.sync.dma_start(
                    out[ic * 512:(ic + 1) * 512, :].rearrange("(s p) d -> p s d", p=128),
                    o_sb[:],
                )
    nc.compile()
    return nc


PERM = np.concatenate([np.arange(0, 128), np.arange(128, 256),
                       np.arange(384, 512), np.arange(256, 384)])


def make_in_maps(node_embedding, adjacency_matrix, conv_w, conv_b, w_ih, w_hh, b_ih, b_hh):
    X = np.asarray(node_embedding, dtype=np.float32)
    A = np.asarray(adjacency_matrix, dtype=np.float32)
    wih_p = np.asarray(w_ih, dtype=np.float32)[PERM]
    whh_p = np.asarray(w_hh, dtype=np.float32)[PERM]
    bias_p = (np.asarray(b_ih, dtype=np.float32) + np.asarray(b_hh, dtype=np.float32))[PERM]
    S = wih_p.sum(axis=1)

    cwt = np.asarray(conv_w, dtype=np.float32).transpose(2, 1, 0)  # [3,128,128]
    packg = np.zeros((2, 640), np.float32)
    packg[:, 0:512] = np.stack([bias_p, S]).reshape(2, 4, 128).reshape(2, 512)
    packg[0, 512:640] = 1.0
    packg[1, 512:640] = np.asarray(conv_b, np.float32)

    packb = np.zeros((128, 641), ml_dtypes.bfloat16)
    packb[:, 0:128] = np.eye(128, dtype=ml_dtypes.bfloat16)
    packb[:, 128] = 1.0
    packb[:, 129:641] = wih_p.T.astype(ml_dtypes.bfloat16)

    zeros1 = np.zeros((128,), np.float32)
    ones1 = np.ones((128,), np.float32)

    in_maps = []
    for c in range(8):
        b, h = c // 2, c % 2
        packc = np.zeros((128, 512), np.float32)
        packc[:, 0:128] = X[b, N - HID:, :]
        packc[:, 128:512] = np.concatenate([cwt[0], cwt[1], cwt[2]], axis=1)
        packc = packc.astype(ml_dtypes.bfloat16)
        packf = np.zeros((128, 642), np.float32)
        packf[:, 0:512] = whh_p.T
        packf[:, 512:640] = np.eye(128, dtype=np.float32)
        packf[:, 640] = ones1 if h == 1 else zeros1
        packf[:, 641] = zeros1 if h == 1 else ones1

        jorder = np.concatenate([np.arange(h * HB, (h + 1) * HB),
                                 np.arange((1 - h) * HB, (2 - h) * HB)])
        Ah = A[b, h * HALF:(h + 1) * HALF, :]
        AT = np.ascontiguousarray(Ah.T)
        atf = AT.reshape(NB, 128, HALF)[jorder].transpose(1, 0, 2)
        xb = X[b].reshape(NB, 128, HID)[jorder].transpose(1, 0, 2)
        m = {
            "packc": packc,
            "packf": packf,
            "packb": packb,
            "packg": packg,
            "atf8": np.ascontiguousarray(atf).astype(NP_FP8),
            "xbf": np.ascontiguousarray(xb).astype(ml_dtypes.bfloat16),
        }
        in_maps.append(m)
    return in_maps


def kernel(node_embedding, adjacency_matrix, conv_w, conv_b, w_ih, w_hh, b_ih, b_hh):
    if "nc" not in _CACHED:
        _CACHED["nc"] = build_nc()
    nc = _CACHED["nc"]
    in_maps = make_in_maps(node_embedding, adjacency_matrix, conv_w, conv_b,
                           w_ih, w_hh, b_ih, b_hh)
    _CACHED["in_maps"] = in_maps
    res = bass_utils.run_bass_kernel_spmd(nc, in_maps, core_ids=list(range(8)))
    out = np.empty((B, N, HID), np.float32)
    for c in range(8):
        b, h = c // 2, c % 2
        out[b, h * HALF:(h + 1) * HALF, :] = res.results[c]["out"]
    return out
